# revision 58
# baseline (speedup 1.0000x reference)
"""Causal multi-head attention on 8 Trainium2 NeuronCores.

Problem: B=4, S=2048, D=1024, H=16 heads, d_k=64, causal, fp32 in/out.

Sharding (host side): core c handles batch b=c//2 and head-half hh=c%2
(8 heads = 512 of the 1024 model dims). Each core computes its batch's
attention output for its 8 heads and the partial out-projection through
the matching 512 rows of Wo (+ bo/2, so the pair sums to +bo). The host
gathers by summing the two partials per batch. No collectives needed.

Final design (560us baseline -> ~310us):
 - all-bf16 PE path (FWL weight loads; fp8 tried and rejected: 3.5e-2
   rel err vs the 2e-2 gate), PSUM accumulates fp32, exact causal trims
 - single j-major emission stream: chunk-j attention interleaved with
   chunk-(j+1) projections and ~2-chunk-delayed out-projections, so
   the list scheduler always has ready PE work (keeps HAM at 2.4GHz)
 - PSUM: scores 2x[128,1024] (4 banks) + av 2x[65,512] + shared
   proj/outproj ring 2x[128,512] = 8 banks exactly
 - ONE big DMA per input tensor, first-task slices first (SP issues
   descriptors at ~640ns each - many small DMAs serialize the start)
 - softmax reciprocal via reciprocal_approx_fast on an SBUF-staged
   copy (InstReciprocal on [1,512] costs 3.35us; approx is ~5x faster;
   PSUM input to the custom-DVE op silently returns garbage)

On-core layout:
  xT  [1024, 2048]  x[b]^T  bf16                  (host-transposed)
  Q^T, K^T [512, 2048] as 4 tiles [128, 2048]     (head pair per tile)
  V   16 tiles [128 keys, 8 heads x 65] bf16      (65th col = ones -> rowsums)
  scores S^T[k, q] = K^T.T @ Q^T  (contraction d=64; head A at partitions
        0-63, head B at 64-127 -> disjoint PE row groups)
  P^T = exp(0.125 * (S^T + causal mask)) on ACT, straight from PSUM
  AV: out^T[65, q] += V_ext[kb].T @ P^T[kb]  (k-blocks, causally trimmed)
  normalize: row 64 = rowsum -> reciprocal_approx_fast -> gpsimd
        partition_broadcast -> multiply into A^T tiles
  out[s, dm] = A^T.T @ Wo_local + bo/2
"""
import sys

for _p in ("/opt/trn_rl_repo",):
    if _p not in sys.path:
        sys.path.insert(0, _p)

import numpy as np

import concourse.bass as bass
import concourse.tile as tile
from concourse import bacc, bass_utils, library_config, mybir

F32 = mybir.dt.float32
BF16 = mybir.dt.bfloat16
EXPF = mybir.ActivationFunctionType.Exp
ADD = mybir.AluOpType.add
MULT = mybir.AluOpType.mult

D = 1024          # model dim
S = 2048          # sequence length
DL = 512          # local head dims (8 heads x 64)
NH = 8            # local heads
NC_ = 8           # cores
NEG = -1.0e30

_CACHE = {}
TRACE = False
last_results = None


def build_program():
    nc = bacc.Bacc("TRN2", target_bir_lowering=False, debug=False)

    # inputs pre-chunked on host: [p, c, n] = full[128c + p, n] so each
    # tensor loads with ONE big DMA (SP descriptor issue is ~640ns each;
    # 57 small DMAs cost ~35us of issue serialization)
    xt_d = nc.dram_tensor("xt", [128, 8, S], BF16, kind="ExternalInput").ap()
    wq_d = nc.dram_tensor("wq", [128, 8, DL], BF16, kind="ExternalInput").ap()
    wk_d = nc.dram_tensor("wk", [128, 8, DL], BF16, kind="ExternalInput").ap()
    wv_d = nc.dram_tensor("wv", [128, 8, DL], BF16, kind="ExternalInput").ap()
    wo_d = nc.dram_tensor("wo", [128, 4, D], BF16, kind="ExternalInput").ap()
    bq_d = nc.dram_tensor("bq2", [128, 4], F32, kind="ExternalInput").ap()
    bk_d = nc.dram_tensor("bk2", [128, 4], F32, kind="ExternalInput").ap()
    bv_d = nc.dram_tensor("bv", [DL], F32, kind="ExternalInput").ap()
    bo_d = nc.dram_tensor("boh", [D], F32, kind="ExternalInput").ap()
    tri_d = nc.dram_tensor("tri", [128, 128], F32, kind="ExternalInput").ap()
    out_d = nc.dram_tensor("out", [S, D], F32, kind="ExternalOutput").ap()

    with tile.TileContext(nc) as tc:
        consts = tc.alloc_tile_pool(name="consts", bufs=1)

        # ---- constant tiles (DMAs emitted AFTER the critical inputs;
        # the library-load and consts DMAs were stealing queue priority
        # from the first projection's 1.5MB) ----
        tri = consts.tile([128, 128], F32, tag="tri", name="tri")
        bq2 = consts.tile([128, 4], F32, tag="bq2", name="bq2")
        bk2 = consts.tile([128, 4], F32, tag="bk2", name="bk2")
        bvb = consts.tile([128, DL], F32, tag="bvb", name="bvb")
        bob = consts.tile([128, D], F32, tag="bob", name="bob")
        ones8 = consts.tile([128, NH], F32, tag="ones8", name="ones8")

        # ---- persistent data pools (all live together; no phase bars) --
        xtp = tc.alloc_tile_pool(name="xtp", bufs=1)
        xtall = xtp.tile([128, 8, S], BF16, tag="xt", name="xt")
        xt = [xtall[:, i, :] for i in range(8)]
        wqp = tc.alloc_tile_pool(name="wqp", bufs=1)
        wqall = wqp.tile([128, 8, DL], BF16, tag="wq", name="wq")
        wqt = [wqall[:, i, :] for i in range(8)]
        wkp = tc.alloc_tile_pool(name="wkp", bufs=1)
        wkall = wkp.tile([128, 8, DL], BF16, tag="wk", name="wk")
        wkt = [wkall[:, i, :] for i in range(8)]
        wvp = tc.alloc_tile_pool(name="wvp", bufs=1)
        wvall = wvp.tile([128, 8, DL], BF16, tag="wv", name="wv")
        wvt = [wvall[:, i, :] for i in range(8)]
        qkp = tc.alloc_tile_pool(name="qkp", bufs=1)
        qt = [qkp.tile([128, S], BF16, tag=f"qt{i}", name=f"qt{i}")
              for i in range(4)]
        kt = [qkp.tile([128, S], BF16, tag=f"kt{i}", name=f"kt{i}")
              for i in range(4)]
        vp = tc.alloc_tile_pool(name="vp", bufs=1)
        v = [vp.tile([128, NH, 65], BF16, tag=f"v{i}", name=f"v{i}")
             for i in range(16)]
        atp = tc.alloc_tile_pool(name="atp", bufs=1)
        at = [atp.tile([128, S], BF16, tag=f"at{i}", name=f"at{i}")
              for i in range(4)]
        wop = tc.alloc_tile_pool(name="wop", bufs=1)
        woall = wop.tile([128, 4, D], BF16, tag="wo", name="wo")
        wo = [woall[:, i, :] for i in range(4)]
        ptp = tc.alloc_tile_pool(name="ptp", bufs=24)
        rcp = tc.alloc_tile_pool(name="rcp", bufs=2)
        bcp = tc.alloc_tile_pool(name="bcp", bufs=2)
        outp = tc.alloc_tile_pool(name="outp", bufs=3)

        # PSUM: s4p 2x2 banks + avp 2x1 + auxp 2x1 = 8 banks. proj and
        # outproj share auxp, but outproj(j-1) is emitted AFTER
        # chunkproj(j+1) so ring predecessors are always ready-to-drain.
        s4p = tc.alloc_tile_pool(name="s4p", bufs=2, space="PSUM")
        avp = tc.alloc_tile_pool(name="avp", bufs=2, space="PSUM")
        auxp = tc.alloc_tile_pool(name="auxp", bufs=2, space="PSUM")

        # ---- input DMAs: task-(0,0) deps first (xt cols 0:512 + the
        # dc=0 slices of Wq/Wk), then small consts, then the rest ----
        nc.sync.dma_start(xtall[:, :, 0:512], xt_d[:, :, 0:512])
        nc.sync.dma_start(wqall[:, :, 0:128], wq_d[:, :, 0:128])
        nc.sync.dma_start(wkall[:, :, 0:128], wk_d[:, :, 0:128])
        nc.sync.dma_start(bq2, bq_d)
        nc.sync.dma_start(bk2, bk_d)
        nc.sync.dma_start(tri, tri_d)
        nc.sync.dma_start(wqall[:, :, 128:DL], wq_d[:, :, 128:DL])
        nc.sync.dma_start(wkall[:, :, 128:DL], wk_d[:, :, 128:DL])
        nc.sync.dma_start(wvall[:], wv_d)
        nc.sync.dma_start(xtall[:, :, 512:S], xt_d[:, :, 512:S])
        nc.sync.dma_start(woall[:], wo_d)
        nc.gpsimd.dma_start(
            bvb,
            bass.AP(tensor=bv_d.tensor, offset=bv_d.offset,
                    ap=[[0, 128]] + bv_d.ap))
        nc.gpsimd.dma_start(
            bob,
            bass.AP(tensor=bo_d.tensor, offset=bo_d.offset,
                    ap=[[0, 128]] + bo_d.ap))
        nc.vector.memset(ones8[:], 1.0)
        nc.gpsimd.load_library(library_config.attn)

        bvb3 = bvb[:].rearrange("p (h d) -> p h d", h=NH)

        # ================= emitters ==================================
        def emit_qk_proj(j, dc):
            for wts, b2, dst in ((wqt, bq2, qt), (wkt, bk2, kt)):
                ps = auxp.tile([128, 512], F32, tag="aux", name="psqk")
                for c in range(8):
                    nc.tensor.matmul(
                        ps[:],
                        wts[c][:, dc * 128:(dc + 1) * 128],
                        xt[c][:, j * 512:(j + 1) * 512],
                        start=(c == 0), stop=(c == 7))
                nc.vector.tensor_scalar_add(
                    dst[dc][:, j * 512:(j + 1) * 512],
                    ps[:], b2[:, dc:dc + 1])

        def emit_v_proj(sb):
            ps = auxp.tile([128, 512], F32, tag="aux", name="psv")
            for c in range(8):
                nc.tensor.matmul(
                    ps[:],
                    xt[c][:, sb * 128:(sb + 1) * 128],
                    wvt[c][:],
                    start=(c == 0), stop=(c == 7))
            nc.vector.tensor_tensor(
                v[sb][:, :, 0:64],
                ps[:].rearrange("p (h d) -> p h d", h=NH),
                bvb3, op=ADD)
            nc.vector.tensor_copy(v[sb][:, :, 64], ones8[:])

        def emit_chunk_proj(j):
            for dc in range(4):
                emit_qk_proj(j, dc)
            for sb in range(4 * j, 4 * j + 4):
                emit_v_proj(sb)

        def emit_outproj(j):
            for sb in range(4 * j, 4 * j + 4):
                ot = outp.tile([128, D], F32, tag="ot", name="ot")
                for n in range(2):
                    ps = auxp.tile([128, 512], F32, tag="aux", name="psd")
                    for hc in range(4):
                        nc.tensor.matmul(
                            ps[:],
                            at[hc][:, sb * 128:(sb + 1) * 128],
                            wo[hc][:, n * 512:(n + 1) * 512],
                            start=(hc == 0), stop=(hc == 3))
                    nc.vector.tensor_tensor(
                        ot[:, n * 512:(n + 1) * 512], ps[:],
                        bob[:, n * 512:(n + 1) * 512], op=ADD)
                nc.sync.dma_start(out_d[sb * 128:(sb + 1) * 128, :], ot[:])

        def emit_pass1(dc, j):
            """Scores + exp for all k-blocks of q-chunk j; returns pt tiles."""
            pts = {}              # (g, half) -> pt tile (bf16)
            for g in range(2 * j + 2):   # kb-groups of 2
                s4s = {}
                cs_list = []
                for kk in range(2):
                    kb = 2 * g + kk
                    cs = max(0, 128 * kb - 512 * j)   # exact causal trim
                    cs_list.append((kb, cs))
                # scores: interleave halves so A (rows 0-63) and B
                # (rows 64-127) can overlap in disjoint PE row groups
                for half in range(2):
                    s4s[half] = s4p.tile([128, 1024], F32,
                                         tag="s4", name="s4")
                for kk, (kb, cs) in enumerate(cs_list):
                    for half in range(2):
                        pr = 64 * half
                        nc.tensor.matmul(
                            s4s[half][:, 512 * kk + cs:512 * (kk + 1)],
                            kt[dc][pr:pr + 64, 128 * kb:128 * (kb + 1)],
                            qt[dc][pr:pr + 64, 512 * j + cs:512 * (j + 1)],
                            start=True, stop=True)
                for half in range(2):
                    s4 = s4s[half]
                    for kk, (kb, cs) in enumerate(cs_list):
                        if 128 * kb >= 512 * j:     # diagonal block
                            sl = s4[:, 512 * kk + cs:512 * kk + cs + 128]
                            nc.vector.tensor_tensor(sl, sl, tri[:], op=ADD)
                    cs0 = cs_list[0][1]
                    cs1 = cs_list[1][1]
                    pt = ptp.tile([128, 1024], BF16, tag="pt", name="pt")
                    pts[(g, half)] = pt
                    if cs1 < 352:
                        # merged call; [512:512+cs1) is never-read garbage
                        nc.scalar.activation(
                            pt[:, cs0:1024], s4[:, cs0:1024],
                            EXPF, scale=0.125)
                    else:
                        nc.scalar.activation(
                            pt[:, cs0:512], s4[:, cs0:512],
                            EXPF, scale=0.125)
                        nc.scalar.activation(
                            pt[:, 512 + cs1:1024], s4[:, 512 + cs1:1024],
                            EXPF, scale=0.125)
            return pts

        def emit_pass2(dc, j, pts):
            """One long AV accumulation chain per head + normalize."""
            for half in range(2):
                pr = 64 * half
                av = avp.tile([65, 512], F32, tag="av", name="av")
                for g in range(2 * j + 2):
                    pt = pts[(g, half)]
                    for kk in range(2):
                        kb = 2 * g + kk
                        cs = max(0, 128 * kb - 512 * j)
                        first = (g == 0 and kk == 0)
                        nc.tensor.matmul(
                            av[:, cs:512],
                            v[kb][:, 2 * dc + half, :],
                            pt[:, 512 * kk + cs:512 * (kk + 1)],
                            start=first, stop=True,
                            skip_group_check=not first)
                rsum = rcp.tile([1, 512], F32, tag="rsum", name="rsum")
                nc.vector.tensor_copy(rsum[:], av[64:65, :])
                rec = rcp.tile([1, 512], F32, tag="rec", name="rec")
                nc.vector.reciprocal_approx_fast(rec[:], rsum[:])
                bc = bcp.tile([64, 512], F32, tag="bc", name="bc")
                nc.gpsimd.partition_broadcast(bc[:], rec[:])
                nc.vector.tensor_tensor(
                    at[dc][pr:pr + 64, 512 * j:512 * (j + 1)],
                    av[0:64, :], bc[:], op=MULT)

        # ================= schedule ==================================
        # Chunk-0 projections just-in-time: Q/K for dc emitted right
        # before pass1(0,dc) so the first exp isn't gated on the whole
        # chunk-0 projection block. Later chunks' projections are
        # emitted mid-previous-chunk; outproj AFTER them so PE prefers
        # feeding the next chunk's scores pipeline.
        emit_qk_proj(0, 0)
        tasks = [(j, dc) for j in range(4) for dc in range(4)]
        # outproj(pj) is deliberately emitted ~2 chunks late so it acts
        # as PE filler during the ACT-bound late chunks
        out_spot = {0: (2, 1), 1: (3, 1), 2: (3, 3)}
        prev = None
        for j, dc in tasks:
            pts = emit_pass1(dc, j)
            if (j, dc) == (0, 0):
                # rest of chunk-0 projections run during task (0,0);
                # QK(0,1) first so scores(0,1) unblocks soonest
                emit_qk_proj(0, 1)
                for sb in range(4):
                    emit_v_proj(sb)
                emit_qk_proj(0, 2)
                emit_qk_proj(0, 3)
            if prev is not None:
                emit_pass2(*prev)
            if dc == 1 and j < 3:
                emit_chunk_proj(j + 1)
            for pj, spot in out_spot.items():
                if spot == (j, dc):
                    emit_outproj(pj)
            prev = (dc, j, pts)
        emit_pass2(*prev)
        emit_outproj(3)

        auxp.release()
        avp.release()
        s4p.release()
        outp.release()
        bcp.release()
        rcp.release()
        ptp.release()
        wop.release()
        atp.release()
        vp.release()
        qkp.release()
        wvp.release()
        wkp.release()
        wqp.release()
        xtp.release()
        consts.release()

    nc.compile()
    return nc


def make_in_maps(x, Wq, bq, Wk, bk, Wv, bv, Wo, bo):
    from ml_dtypes import bfloat16
    x = np.asarray(x, np.float32)
    Wq, bq = np.asarray(Wq, np.float32), np.asarray(bq, np.float32)
    Wk, bk = np.asarray(Wk, np.float32), np.asarray(bk, np.float32)
    Wv, bv = np.asarray(Wv, np.float32), np.asarray(bv, np.float32)
    Wo, bo = np.asarray(Wo, np.float32), np.asarray(bo, np.float32)

    k = np.arange(128)[:, None]
    c = np.arange(128)[None, :]
    tri = np.where(c >= k, 0.0, NEG).astype(np.float32)
    boh = (bo * 0.5).astype(np.float32)

    def chunked(a):
        """[128*nc, n] -> [128, nc, n] bf16 with [p, c, n] = a[128c+p, n]."""
        nch = a.shape[0] // 128
        return np.ascontiguousarray(
            a.reshape(nch, 128, a.shape[1]).transpose(1, 0, 2).astype(bfloat16))

    in_maps = []
    for core in range(NC_):
        b, hh = core // 2, core % 2
        sl = slice(hh * DL, (hh + 1) * DL)
        in_maps.append({
            "xt": chunked(x[b].T),
            "wq": chunked(Wq[:, sl]),
            "wk": chunked(Wk[:, sl]),
            "wv": chunked(Wv[:, sl]),
            "wo": chunked(Wo[sl, :]),
            "bq2": np.ascontiguousarray(bq[sl].reshape(4, 128).T),
            "bk2": np.ascontiguousarray(bk[sl].reshape(4, 128).T),
            "bv": np.ascontiguousarray(bv[sl]),
            "boh": boh,
            "tri": tri,
        })
    return in_maps


def kernel(x, Wq, bq, Wk, bk, Wv, bv, Wo, bo):
    global last_results
    if "nc" not in _CACHE:
        _CACHE["nc"] = build_program()
    nc = _CACHE["nc"]
    in_maps = make_in_maps(x, Wq, bq, Wk, bk, Wv, bv, Wo, bo)
    res = bass_utils.run_bass_kernel_spmd(
        nc, in_maps, core_ids=list(range(NC_)), trace=TRACE)
    last_results = res
    B = 4
    out = np.empty((B, S, D), np.float32)
    for b in range(B):
        out[b] = res.results[2 * b]["out"] + res.results[2 * b + 1]["out"]
    return out
